# revision 1
# baseline (speedup 1.0000x reference)
"""TRN2 Bass kernel for nn_Attention_23493471109551.

Full attention layer: QKV projections + interleaved RoPE + causal softmax
attention + output projection, for B=4, S=2048, D=1024, H=16, Dh=64, fp32.

Sharding: 8 cores = 4 batches x 2 head-groups (8 heads each).  Each core
computes its batch/head-group's attention and a partial output projection
(W_o row-block); host sums the two partials per batch.

All matmuls run in fp32r (fp32 rounded to 11-bit mantissa, 1 cyc/row on the
PE at N>=512 vs 4 for fp32).  Inputs are pre-rounded on host; on-device
intermediates are rounded by the producing engine writing float32r tiles.

Layout strategy (per core):
  qpT/kpT: [dh-on-partitions, S]  (heads stacked 2-per-128-partitions)
  scores computed TRANSPOSED [sk, sq] so probs feed the PV matmul directly
  (no per-tile transposes); softmax denominator comes free as a ones-column
  appended to V (fused into the PV matmul, normalized once per [65,512]
  output block); causal mask is an additive -1e30 on the score PSUM applied
  only to the diagonal 128x128 blocks, with score/exp/PV column ranges
  trimmed to the causal triangle; RoPE pair-mixing uses a DVE stream-shuffle.

Measured (8 cores, NTFF profile): ~800-880 us per core, rel err ~1.8e-4.
"""
import math
import numpy as np

import concourse.bass as bass
import concourse.tile as tile
import concourse.mybir as mybir
from concourse import bacc, bass_utils

# problem constants
B, S, D = 4, 2048, 1024
H, Dh = 16, 64
EQ, EV = 2048, 1024          # q/k and v input feature dims
F = 512                      # features per core (8 heads x 64)
P = 128
N_CORES = 8
SCALE = 1.0 / math.sqrt(D)   # 1/32
ROPE_BASE = 10000.0
NEG = -1.0e30
SWAP_MASK = [i ^ 1 for i in range(32)]

F32 = mybir.dt.float32
F32R = mybir.dt.float32r

# test hooks (harness ignores these)
KERNEL_TRACE = False
LAST_RESULT = None

_nc_cache = None


def _round_fp32r(x: np.ndarray) -> np.ndarray:
    """Round fp32 array to the fp32r grid (11-bit mantissa, RNE)."""
    u = np.ascontiguousarray(x, dtype=np.float32).view(np.uint32)
    low = u & np.uint32(0xFFF)
    hi = u >> np.uint32(12)
    round_up = (low > np.uint32(0x800)) | (
        (low == np.uint32(0x800)) & ((hi & np.uint32(1)) == 1)
    )
    hi = hi + round_up.astype(np.uint32)
    return (hi << np.uint32(12)).view(np.float32)


def _build_nc():
    nc = bacc.Bacc("TRN2", target_bir_lowering=False, debug=False)
    qT = nc.dram_tensor("qT", [EQ, S], F32R, kind="ExternalInput").ap()
    kT = nc.dram_tensor("kT", [EQ, S], F32R, kind="ExternalInput").ap()
    vT = nc.dram_tensor("vT", [EV, S], F32R, kind="ExternalInput").ap()
    wqT = nc.dram_tensor("wqT", [EQ, F], F32R, kind="ExternalInput").ap()
    wkT = nc.dram_tensor("wkT", [EQ, F], F32R, kind="ExternalInput").ap()
    wvT = nc.dram_tensor("wvT", [EV, F], F32R, kind="ExternalInput").ap()
    woT = nc.dram_tensor("woT", [F, D], F32R, kind="ExternalInput").ap()
    cosf = nc.dram_tensor("cosf", [P, S], F32, kind="ExternalInput").ap()
    sinf = nc.dram_tensor("sinf", [P, S], F32, kind="ExternalInput").ap()
    maskA = nc.dram_tensor("maskA", [P, P], F32, kind="ExternalInput").ap()
    out = nc.dram_tensor("out", [S, D], F32, kind="ExternalOutput").ap()

    EXP = mybir.ActivationFunctionType.Exp

    with tile.TileContext(nc) as tc:
        with (
            tc.tile_pool(name="consts", bufs=1) as consts,
            tc.tile_pool(name="persist", bufs=1) as persist,
            tc.tile_pool(name="qt", bufs=5) as qt_pool,
            tc.tile_pool(name="wsmall", bufs=6) as w_pool,
            tc.tile_pool(name="rope", bufs=2) as rope_pool,
            tc.tile_pool(name="expp", bufs=3) as exp_pool,
            tc.tile_pool(name="norm", bufs=2) as norm_pool,
            tc.tile_pool(name="outsb", bufs=3) as out_pool,
            tc.tile_pool(name="attnc", bufs=2) as attnc_pool,
            tc.tile_pool(name="ps1", bufs=4, space="PSUM") as ps1,
            tc.tile_pool(name="ps2", bufs=2, space="PSUM") as ps2,
        ):
            # ---- persistent activations
            qpT = persist.tile([P, 4, S], F32R, tag="qpT")
            kpT = persist.tile([P, 4, S], F32R, tag="kpT")
            vpa = persist.tile([P, 16, 8, 65], F32R, tag="vpa")

            nc.vector.memset(vpa[:].bitcast(F32), 1.0)  # ones col; 0:64 overwritten

            # ---- constants (emitted after the first proj tiles get queue priority)
            cos_t = consts.tile([P, S], F32, tag="cos")
            sin_t = consts.tile([P, S], F32, tag="sin")
            mask_t = consts.tile([P, P], F32, tag="mask")
            wo_t = consts.tile([P, 4, D], F32R, tag="woT")

            # ---- q/k projections + rope (qpT[f, s] = sum_e WT[e,f] * xT[e,s])
            _const_dmas_emitted = False
            for src, wsrc, dstT in ((qT, wqT, qpT), (kT, wkT, kpT)):
                for sc in range(4):
                    ssl = slice(sc * 512, (sc + 1) * 512)
                    psums = [None] * 4
                    for e in range(16):
                        at = qt_pool.tile([P, 512], F32R, tag="qt")
                        nc.sync.dma_start(at[:], src[e * P:(e + 1) * P, ssl])
                        wt = w_pool.tile([P, F], F32R, tag="w")
                        nc.sync.dma_start(wt[:], wsrc[e * P:(e + 1) * P, :])
                        for ci in range(4):
                            if psums[ci] is None:
                                psums[ci] = ps1.tile([P, 512], F32, tag="b1",
                                                     name=f"psq{ci}")
                            nc.tensor.matmul(psums[ci][:], wt[:, ci * P:(ci + 1) * P],
                                             at[:], start=(e == 0), stop=(e == 15))
                    if not _const_dmas_emitted:
                        _const_dmas_emitted = True
                        nc.gpsimd.dma_start(cos_t[:], cosf)
                        nc.gpsimd.dma_start(sin_t[:], sinf)
                        nc.gpsimd.dma_start(mask_t[:], maskA)
                        for _ci in range(4):
                            nc.gpsimd.dma_start(wo_t[:, _ci, :],
                                                woT[_ci * P:(_ci + 1) * P, :])
                    # rope: out = x*cos + pairswap(x*sin')
                    for ci in range(4):
                        a_t = rope_pool.tile([P, 512], F32, tag="ropeA")
                        nc.vector.tensor_mul(a_t[:], psums[ci][:], cos_t[:, ssl])
                        c_t = rope_pool.tile([P, 512], F32, tag="ropeC")
                        nc.vector.tensor_mul(c_t[:], psums[ci][:], sin_t[:, ssl])
                        c2_t = rope_pool.tile([P, 512], F32, tag="ropeC")
                        nc.vector.stream_shuffle(c2_t[:], c_t[:], SWAP_MASK)
                        nc.vector.tensor_add(dstT[:, ci, ssl], a_t[:], c2_t[:])

            # ---- v projection (vp[s, f] = sum_e vT[e,s] * wvT[e,f])
            for stq in range(4):
                psv = [ps1.tile([P, 512], F32, tag="b1", name=f"psv{i}") for i in range(4)]
                for e in range(8):
                    wvt = w_pool.tile([P, F], F32R, tag="w")
                    nc.sync.dma_start(wvt[:], wvT[e * P:(e + 1) * P, :])
                    vt = w_pool.tile([P, F], F32R, tag="w", name="vtrow")
                    nc.sync.dma_start(
                        vt[:], vT[e * P:(e + 1) * P, stq * 512:(stq + 1) * 512])
                    for j in range(4):
                        nc.tensor.matmul(psv[j][:], vt[:, j * P:(j + 1) * P], wvt[:],
                                         start=(e == 0), stop=(e == 7))
                for j in range(4):
                    st = stq * 4 + j
                    nc.scalar.copy(vpa[:, st, :, 0:64],
                                   psv[j][:].rearrange("p (h d) -> p h d", h=8))

            # ---- attention (scoresT layout) + output projection, per s-chunk
            for c in range(4):
                ssl = slice(c * 512, (c + 1) * 512)
                nt = 4 * (c + 1)
                attn_c = attnc_pool.tile([P, 4, 512], F32R, tag="attn_c")
                for hp in range(4):
                    po_a = ps1.tile([P, 512], F32, tag="b1")
                    po_b = ps1.tile([P, 512], F32, tag="b1")
                    for t in range(nt):
                        tsl = slice(t * P, (t + 1) * P)
                        rr = 128 * (t - 4 * c) if t >= 4 * c else 0
                        qsl = slice(c * 512 + rr, (c + 1) * 512)
                        ps_s = ps2.tile([P, 1024], F32, tag="b2")
                        nc.tensor.matmul(ps_s[:, rr:512], kpT[0:64, hp, tsl],
                                         qpT[0:64, hp, qsl], start=True, stop=True)
                        nc.tensor.matmul(ps_s[:, 512 + rr:1024], kpT[64:128, hp, tsl],
                                         qpT[64:128, hp, qsl], start=True, stop=True)
                        sv = ps_s[:].rearrange("p (two n) -> p two n", two=2)
                        if t >= 4 * c:
                            nc.vector.tensor_add(
                                sv[:, :, rr:rr + 128], sv[:, :, rr:rr + 128],
                                mask_t[:, None, :].to_broadcast((P, 2, P)))
                        et = exp_pool.tile([P, 2, 512], F32R, tag="exp")
                        nc.scalar.activation(et[:, :, rr:512], sv[:, :, rr:512],
                                             EXP, scale=SCALE)
                        nc.tensor.matmul(po_a[0:65, rr:512], vpa[:, t, 2 * hp, :],
                                         et[:, 0, rr:512], start=(t == 0), stop=(t == nt - 1))
                        nc.tensor.matmul(po_b[0:65, rr:512], vpa[:, t, 2 * hp + 1, :],
                                         et[:, 1, rr:512], start=(t == 0), stop=(t == nt - 1))
                    # normalize by the ones-column sum (row 64)
                    for half, po in ((0, po_a), (1, po_b)):
                        posb = norm_pool.tile([65, 512], F32, tag="posb")
                        nc.scalar.copy(posb[:], po[0:65, :])
                        rc = norm_pool.tile([1, 512], F32, tag="recip")
                        nc.vector.reciprocal(rc[:], posb[64:65, :])
                        bc = norm_pool.tile([64, 512], F32, tag="bcast")
                        nc.gpsimd.partition_broadcast(bc[:], rc[:])
                        nc.vector.tensor_mul(
                            attn_c[64 * half:64 * (half + 1), hp, :],
                            posb[0:64, :], bc[:])
                # W_o for this chunk's 4 s-tiles
                for j in range(4):
                    pw = [ps1.tile([P, 512], F32, tag="b1", name=f"pw{i}") for i in range(2)]
                    for ci in range(4):
                        for oc in range(2):
                            nc.tensor.matmul(pw[oc][:], attn_c[:, ci, j * P:(j + 1) * P],
                                             wo_t[:, ci, oc * 512:(oc + 1) * 512],
                                             start=(ci == 0), stop=(ci == 3))
                    row = (4 * c + j) * P
                    for oc in range(2):
                        ot = out_pool.tile([P, 512], F32, tag="osb")
                        nc.vector.tensor_copy(ot[:], pw[oc][:])
                        nc.sync.dma_start(out[row:row + P, oc * 512:(oc + 1) * 512],
                                          ot[:])
    nc.compile()
    return nc


def _tables():
    inv = (1.0 / (ROPE_BASE ** (np.arange(0, Dh, 2, dtype=np.float32) / Dh))
           ).astype(np.float32)                      # [32]
    pos = np.arange(S, dtype=np.float32)
    ang = pos[:, None] * inv[None, :]                # [S, 32]
    cos = np.cos(ang).astype(np.float32)
    sin = np.sin(ang).astype(np.float32)
    d = np.arange(P) % Dh
    i = d // 2
    cosf = np.ascontiguousarray(cos[:, i].T)         # [128, S]
    sgn = np.where(d % 2 == 0, 1.0, -1.0).astype(np.float32)
    sinf = np.ascontiguousarray(sin[:, i].T * sgn[:, None]).astype(np.float32)

    p = np.arange(P)
    j = np.arange(P)
    maskA = np.where(p[:, None] <= j[None, :], 0.0, NEG).astype(np.float32)
    return cosf, sinf, maskA


def kernel(q, k, v, W_q, W_k, W_v, W_o):
    global _nc_cache, LAST_RESULT
    if _nc_cache is None:
        _nc_cache = _build_nc()
    nc = _nc_cache

    cosf, sinf, maskA = _tables()
    q = np.asarray(q, dtype=np.float32)
    k = np.asarray(k, dtype=np.float32)
    v = np.asarray(v, dtype=np.float32)
    W_q = np.asarray(W_q, dtype=np.float32)
    W_k = np.asarray(W_k, dtype=np.float32)
    W_v = np.asarray(W_v, dtype=np.float32)
    W_o = np.asarray(W_o, dtype=np.float32)

    in_maps = []
    for b in range(B):
        qTb = _round_fp32r(q[b].T)
        kTb = _round_fp32r(k[b].T)
        vTb = _round_fp32r(v[b].T)
        for g in range(2):
            fs = slice(g * F, (g + 1) * F)
            in_maps.append({
                "qT": qTb, "kT": kTb, "vT": vTb,
                "wqT": _round_fp32r(W_q[fs, :].T),
                "wkT": _round_fp32r(W_k[fs, :].T),
                "wvT": _round_fp32r(W_v[fs, :].T),
                "woT": _round_fp32r(W_o[:, fs].T),
                "cosf": cosf, "sinf": sinf, "maskA": maskA,
            })

    res = bass_utils.run_bass_kernel_spmd(
        nc, in_maps, core_ids=list(range(N_CORES)), trace=KERNEL_TRACE)
    LAST_RESULT = res

    final = np.empty((B, S, D), dtype=np.float32)
    for b in range(B):
        final[b] = res.results[2 * b]["out"] + res.results[2 * b + 1]["out"]
    return final



# revision 9
# speedup vs baseline: 1.2920x; 1.2920x over previous
"""TRN2 Bass kernel for nn_Attention_23493471109551 (v2, fp16).

Full attention layer: QKV projections + interleaved RoPE + causal softmax
attention + output projection, for B=4, S=2048, D=1024, H=16, Dh=64, fp32 I/O.

Sharding: 8 cores = 4 batches x 2 head-groups (8 heads each).  Each core
computes its batch/head-group's attention and a partial output projection
(W_o row-block); host sums the two partials per batch.

v2 changes vs v1 (fp32r baseline, 707us):
  - all matmul operands fp16 (err 3.4e-4 vs 2e-2 budget): halves HBM traffic,
    removes the fp32r n<256 4x penalty, faster ldweights.
  - weights hoisted to SBUF once (v1 reloaded W_q/W_k 4x: 25MB extra DMA).
  - causal mask applied as a 0/1 fp16 multiply on the exp output (SBUF)
    instead of -1e30 add on the score PSUM: cheaper and shortens the
    PSUM critical path.
  - softmax denominator reciprocal via reciprocal_approx_fast (v1's
    nc.vector.reciprocal was 3.3us per call, 106us total DVE).
  - per-chunk emission interleave: projection chunk c+1 is emitted between
    attention chunk c and its output projection, so the tile scheduler can
    fill the (Activation-bound) attention phase with projection matmuls and
    keep the PE continuously busy at its top p-state.

Layout (per core):
  qpT/kpT: [dh-on-partitions (2 heads x 64), hp, S] fp16
  scoresT [sk, sq] in PSUM; exp'd (scale fused) to fp16 et; PV feeds from et
  directly; denominator = ones-column appended to V (row 64 of the PV psum);
  normalization = approx-reciprocal + gpsimd partition_broadcast + fp16 mul.
"""
import math
import numpy as np

import concourse.bass as bass
import concourse.tile as tile
import concourse.mybir as mybir
from concourse import bacc, bass_utils

# problem constants
B, S, D = 4, 2048, 1024
H, Dh = 16, 64
EQ, EV = 2048, 1024          # q/k and v input feature dims
F = 512                      # features per core (8 heads x 64)
P = 128
N_CORES = 8
SCALE = 1.0 / math.sqrt(D)   # 1/32
ROPE_BASE = 10000.0
SWAP_MASK = [i ^ 1 for i in range(32)]

F16 = mybir.dt.float16
F32 = mybir.dt.float32

# test hooks (harness ignores these)
KERNEL_TRACE = False
LAST_RESULT = None

_nc_cache = None


def _build_nc():
    nc = bacc.Bacc("TRN2", target_bir_lowering=False, debug=False)
    qT = nc.dram_tensor("qT", [EQ, S], F16, kind="ExternalInput").ap()
    kT = nc.dram_tensor("kT", [EQ, S], F16, kind="ExternalInput").ap()
    vT = nc.dram_tensor("vT", [EV, S], F16, kind="ExternalInput").ap()
    wqT = nc.dram_tensor("wqT", [EQ, F], F16, kind="ExternalInput").ap()
    wkT = nc.dram_tensor("wkT", [EQ, F], F16, kind="ExternalInput").ap()
    wvT = nc.dram_tensor("wvT", [EV, F], F16, kind="ExternalInput").ap()
    woT = nc.dram_tensor("woT", [F, D], F16, kind="ExternalInput").ap()
    cosf = nc.dram_tensor("cosf", [P, S], F16, kind="ExternalInput").ap()
    sinf = nc.dram_tensor("sinf", [P, S], F16, kind="ExternalInput").ap()
    maskA = nc.dram_tensor("maskA", [P, P], F16, kind="ExternalInput").ap()
    out = nc.dram_tensor("out", [S, D], F32, kind="ExternalOutput").ap()

    EXP = mybir.ActivationFunctionType.Exp

    with tile.TileContext(nc) as tc:
        with (
            tc.tile_pool(name="consts", bufs=1) as consts,
            tc.tile_pool(name="persist", bufs=1) as persist,
            tc.tile_pool(name="insb", bufs=2) as insb_pool,
            tc.tile_pool(name="vsb", bufs=2) as vsb_pool,
            tc.tile_pool(name="rope", bufs=2) as rope_pool,
            tc.tile_pool(name="et", bufs=3) as et_pool,
            tc.tile_pool(name="norm", bufs=2) as norm_pool,
            tc.tile_pool(name="denp", bufs=1) as den_pool,
            tc.tile_pool(name="attnc", bufs=2) as attnc_pool,
            tc.tile_pool(name="outsb", bufs=2) as out_pool,
            tc.tile_pool(name="projps", bufs=2, space="PSUM") as proj_ps,
            tc.tile_pool(name="scps", bufs=2, space="PSUM") as sc_ps,
            tc.tile_pool(name="pops", bufs=2, space="PSUM") as po_ps,
        ):
            # ---- persistent activations
            qpT = persist.tile([P, 4, S], F16, tag="qpT")
            kpT = persist.tile([P, 4, S], F16, tag="kpT")
            vpa = persist.tile([P, 16, 8, 65], F16, tag="vpa")
            nc.vector.memset(vpa[:, :, :, 64:65], 1.0)  # softmax-denominator ones

            # ---- weights + tables, loaded once
            wq_t = consts.tile([P, 16, F], F16, tag="wq")
            wk_t = consts.tile([P, 16, F], F16, tag="wk")
            wv_t = consts.tile([P, 8, F], F16, tag="wv")
            wo_t = consts.tile([P, 4, D], F16, tag="wo")
            cos_t = consts.tile([P, S], F16, tag="cos")
            sin_t = consts.tile([P, S], F16, tag="sin")
            mask_t = consts.tile([P, P], F16, tag="mask")
            for e in range(16):
                nc.gpsimd.dma_start(wq_t[:, e, :], wqT[e * P:(e + 1) * P, :])
                nc.gpsimd.dma_start(wk_t[:, e, :], wkT[e * P:(e + 1) * P, :])
            for e in range(8):
                nc.gpsimd.dma_start(wv_t[:, e, :], wvT[e * P:(e + 1) * P, :])
            for fi in range(4):
                nc.gpsimd.dma_start(wo_t[:, fi, :], woT[fi * P:(fi + 1) * P, :])
            nc.gpsimd.dma_start(cos_t[:], cosf)
            nc.gpsimd.dma_start(sin_t[:], sinf)
            nc.gpsimd.dma_start(mask_t[:], maskA)

            def qk_chunk(c):
                """Project q and k for s-chunk c (fp16 matmuls) + rope."""
                ssl = slice(c * 512, (c + 1) * 512)
                for xT, w_t, dstT, nm in ((qT, wq_t, qpT, "q"),
                                          (kT, wk_t, kpT, "k")):
                    xsb = insb_pool.tile([P, 16, 512], F16, tag="insb",
                                         name=f"{nm}sb{c}")
                    for e in range(16):
                        nc.sync.dma_start(xsb[:, e, :], xT[e * P:(e + 1) * P, ssl])
                    for ci in range(4):
                        ps = proj_ps.tile([P, 512], F32, tag="proj",
                                          name=f"ps{nm}{c}_{ci}")
                        for e in range(16):
                            nc.tensor.matmul(ps[:], w_t[:, e, ci * P:(ci + 1) * P],
                                             xsb[:, e, :],
                                             start=(e == 0), stop=(e == 15))
                        # rope: out = x*cos + pairswap(x*sin')
                        a16 = rope_pool.tile([P, 512], F16, tag="ropeA")
                        nc.vector.tensor_copy(a16[:], ps[:])
                        cm = rope_pool.tile([P, 512], F16, tag="ropeC")
                        nc.vector.tensor_mul(cm[:], a16[:], cos_t[:, ssl])
                        sm = rope_pool.tile([P, 512], F16, tag="ropeS")
                        nc.gpsimd.tensor_mul(sm[:], a16[:], sin_t[:, ssl])
                        sm2 = rope_pool.tile([P, 512], F16, tag="ropeS2")
                        nc.vector.stream_shuffle(sm2[:], sm[:], SWAP_MASK)
                        nc.vector.tensor_add(dstT[:, ci, ssl], cm[:], sm2[:])

            def v_chunk(stq):
                """Project v for s-chunk stq into vpa (s on partitions)."""
                vsb = vsb_pool.tile([P, 8, 512], F16, tag="vsb", name=f"vsb{stq}")
                for e in range(8):
                    nc.sync.dma_start(vsb[:, e, :],
                                      vT[e * P:(e + 1) * P,
                                         stq * 512:(stq + 1) * 512])
                for j in range(4):
                    ps = proj_ps.tile([P, 512], F32, tag="proj",
                                      name=f"psv{stq}_{j}")
                    for e in range(8):
                        nc.tensor.matmul(ps[:], vsb[:, e, j * P:(j + 1) * P],
                                         wv_t[:, e, :],
                                         start=(e == 0), stop=(e == 7))
                    st = stq * 4 + j
                    nc.scalar.copy(vpa[:, st, :, 0:64],
                                   ps[:].rearrange("p (h d) -> p h d", h=8))

            def attn_chunk(c, attn_c):
                """Causal attention for query chunk c (scoresT layout)."""
                nt = 4 * (c + 1)
                for hp in range(4):
                    po_a = po_ps.tile([P, 512], F32, tag="po", name=f"poa{c}_{hp}")
                    po_b = po_ps.tile([P, 512], F32, tag="po", name=f"pob{c}_{hp}")
                    for t in range(nt):
                        tsl = slice(t * P, (t + 1) * P)
                        rr = P * (t - 4 * c) if t >= 4 * c else 0
                        qsl = slice(c * 512 + rr, (c + 1) * 512)
                        ps_s = sc_ps.tile([P, 2, 512], F32, tag="sc",
                                          name=f"scs{c}_{hp}_{t}")
                        nc.tensor.matmul(ps_s[:, 0, rr:512], kpT[0:64, hp, tsl],
                                         qpT[0:64, hp, qsl], start=True, stop=True)
                        nc.tensor.matmul(ps_s[:, 1, rr:512], kpT[64:128, hp, tsl],
                                         qpT[64:128, hp, qsl], start=True, stop=True)
                        et = et_pool.tile([P, 2, 512], F16, tag="et")
                        nc.scalar.activation(et[:, :, rr:512], ps_s[:, :, rr:512],
                                             EXP, scale=SCALE)
                        if t >= 4 * c:
                            # zero the above-diagonal triangle of this block
                            nc.vector.tensor_mul(
                                et[:, :, rr:rr + P], et[:, :, rr:rr + P],
                                mask_t[:, None, :].to_broadcast((P, 2, P)))
                        nc.tensor.matmul(po_a[0:65, rr:512], vpa[:, t, 2 * hp, :],
                                         et[:, 0, rr:512],
                                         start=(t == 0), stop=(t == nt - 1))
                        nc.tensor.matmul(po_b[0:65, rr:512], vpa[:, t, 2 * hp + 1, :],
                                         et[:, 1, rr:512],
                                         start=(t == 0), stop=(t == nt - 1))
                    # normalize: attn = po[0:64] * (1 / po[64]) per head.
                    # All DVE/gpsimd ops keep inputs at base partition 0
                    # (cross-base inputs miscompile; out-offset is fine).
                    denA = den_pool.tile([1, 512], F32, tag="denA")
                    nc.vector.tensor_copy(denA[:], po_a[64:65, :])
                    denB = den_pool.tile([1, 512], F32, tag="denB")
                    nc.vector.tensor_copy(denB[:], po_b[64:65, :])
                    rcfa = den_pool.tile([1, 512], F32, tag="rcfa")
                    nc.vector.reciprocal_approx_fast(out=rcfa[:], in_=denA[:])
                    rcfb = den_pool.tile([1, 512], F32, tag="rcfb")
                    nc.vector.reciprocal_approx_fast(out=rcfb[:], in_=denB[:])
                    bcA = norm_pool.tile([64, 512], F32, tag="bcA")
                    nc.gpsimd.partition_broadcast(bcA[:], rcfa[:])
                    bcB = norm_pool.tile([64, 512], F32, tag="bcB")
                    nc.gpsimd.partition_broadcast(bcB[:], rcfb[:])
                    posbA = norm_pool.tile([64, 512], F16, tag="posbA")
                    nc.vector.tensor_copy(posbA[:], po_a[0:64, :])
                    posbB = norm_pool.tile([64, 512], F16, tag="posbB")
                    nc.vector.tensor_copy(posbB[:], po_b[0:64, :])
                    nc.vector.tensor_mul(attn_c[0:64, hp, :], posbA[:], bcA[:])
                    nc.vector.tensor_mul(attn_c[64:128, hp, :], posbB[:], bcB[:])

            def outproj(c, attn_c):
                for j in range(4):
                    pw = [po_ps.tile([P, 512], F32, tag="po", name=f"pw{c}_{j}_{i}")
                          for i in range(2)]
                    for ci in range(4):
                        for oc in range(2):
                            nc.tensor.matmul(pw[oc][:],
                                             attn_c[:, ci, j * P:(j + 1) * P],
                                             wo_t[:, ci, oc * 512:(oc + 1) * 512],
                                             start=(ci == 0), stop=(ci == 3))
                    row = (4 * c + j) * P
                    for oc in range(2):
                        ot = out_pool.tile([P, 512], F32, tag="ot")
                        nc.vector.tensor_copy(ot[:], pw[oc][:])
                        nc.gpsimd.dma_start(out[row:row + P, oc * 512:(oc + 1) * 512],
                                            ot[:])

            # ---- program: chunk 0 projections, then per chunk: attention,
            # next-chunk projections (scheduler filler for the PE), out-proj.
            qk_chunk(0)
            v_chunk(0)
            for c in range(4):
                attn_c = attnc_pool.tile([P, 4, 512], F16, tag="attn",
                                         name=f"attn{c}")
                attn_chunk(c, attn_c)
                if c < 3:
                    qk_chunk(c + 1)
                    v_chunk(c + 1)
                outproj(c, attn_c)
    nc.compile()
    return nc


def _tables():
    inv = (1.0 / (ROPE_BASE ** (np.arange(0, Dh, 2, dtype=np.float32) / Dh))
           ).astype(np.float32)                      # [32]
    pos = np.arange(S, dtype=np.float32)
    ang = pos[:, None] * inv[None, :]                # [S, 32]
    cos = np.cos(ang).astype(np.float32)
    sin = np.sin(ang).astype(np.float32)
    d = np.arange(P) % Dh
    i = d // 2
    cosf = np.ascontiguousarray(cos[:, i].T).astype(np.float16)   # [128, S]
    sgn = np.where(d % 2 == 0, 1.0, -1.0).astype(np.float32)
    sinf = np.ascontiguousarray(sin[:, i].T * sgn[:, None]).astype(np.float16)

    p = np.arange(P)
    j = np.arange(P)
    maskA = np.where(p[:, None] <= j[None, :], 1.0, 0.0).astype(np.float16)
    return cosf, sinf, maskA


def kernel(q, k, v, W_q, W_k, W_v, W_o):
    global _nc_cache, LAST_RESULT
    if _nc_cache is None:
        _nc_cache = _build_nc()
    nc = _nc_cache

    cosf, sinf, maskA = _tables()
    q = np.asarray(q, dtype=np.float32)
    k = np.asarray(k, dtype=np.float32)
    v = np.asarray(v, dtype=np.float32)
    W_q = np.asarray(W_q, dtype=np.float32)
    W_k = np.asarray(W_k, dtype=np.float32)
    W_v = np.asarray(W_v, dtype=np.float32)
    W_o = np.asarray(W_o, dtype=np.float32)

    in_maps = []
    for b in range(B):
        qTb = np.ascontiguousarray(q[b].T).astype(np.float16)
        kTb = np.ascontiguousarray(k[b].T).astype(np.float16)
        vTb = np.ascontiguousarray(v[b].T).astype(np.float16)
        for g in range(2):
            fs = slice(g * F, (g + 1) * F)
            in_maps.append({
                "qT": qTb, "kT": kTb, "vT": vTb,
                "wqT": np.ascontiguousarray(W_q[fs, :].T).astype(np.float16),
                "wkT": np.ascontiguousarray(W_k[fs, :].T).astype(np.float16),
                "wvT": np.ascontiguousarray(W_v[fs, :].T).astype(np.float16),
                "woT": np.ascontiguousarray(W_o[:, fs].T).astype(np.float16),
                "cosf": cosf, "sinf": sinf, "maskA": maskA,
            })

    res = bass_utils.run_bass_kernel_spmd(
        nc, in_maps, core_ids=list(range(N_CORES)), trace=KERNEL_TRACE)
    LAST_RESULT = res

    final = np.empty((B, S, D), dtype=np.float32)
    for b in range(B):
        final[b] = res.results[2 * b]["out"] + res.results[2 * b + 1]["out"]
    return final


# revision 16
# speedup vs baseline: 1.2997x; 1.0059x over previous
"""TRN2 Bass kernel for nn_Attention_23493471109551 (v2, fp16).

Full attention layer: QKV projections + interleaved RoPE + causal softmax
attention + output projection, for B=4, S=2048, D=1024, H=16, Dh=64, fp32 I/O.

Sharding: 8 cores = 4 batches x 2 head-groups (8 heads each).  Each core
computes its batch/head-group's attention and a partial output projection
(W_o row-block); host sums the two partials per batch.

v2 changes vs v1 (fp32r baseline, 707us):
  - all matmul operands fp16 (err 3.4e-4 vs 2e-2 budget): halves HBM traffic,
    removes the fp32r n<256 4x penalty, faster ldweights.
  - weights hoisted to SBUF once (v1 reloaded W_q/W_k 4x: 25MB extra DMA).
  - causal mask applied as a 0/1 fp16 multiply on the exp output (SBUF)
    instead of -1e30 add on the score PSUM: cheaper and shortens the
    PSUM critical path.
  - softmax denominator reciprocal via reciprocal_approx_fast (v1's
    nc.vector.reciprocal was 3.3us per call, 106us total DVE).
  - per-chunk emission interleave: projection chunk c+1 is emitted between
    attention chunk c and its output projection, so the tile scheduler can
    fill the (Activation-bound) attention phase with projection matmuls and
    keep the PE continuously busy at its top p-state.

Layout (per core):
  qpT/kpT: [dh-on-partitions (2 heads x 64), hp, S] fp16
  scoresT [sk, sq] in PSUM; exp'd (scale fused) to fp16 et; PV feeds from et
  directly; denominator = ones-column appended to V (row 64 of the PV psum);
  normalization = approx-reciprocal + gpsimd partition_broadcast + fp16 mul.
"""
import math
import numpy as np

import concourse.bass as bass
import concourse.tile as tile
import concourse.mybir as mybir
from concourse import bacc, bass_utils

# problem constants
B, S, D = 4, 2048, 1024
H, Dh = 16, 64
EQ, EV = 2048, 1024          # q/k and v input feature dims
F = 512                      # features per core (8 heads x 64)
P = 128
N_CORES = 8
SCALE = 1.0 / math.sqrt(D)   # 1/32
ROPE_BASE = 10000.0
SWAP_MASK = [i ^ 1 for i in range(32)]

F16 = mybir.dt.float16
F32 = mybir.dt.float32

# test hooks (harness ignores these)
KERNEL_TRACE = False
LAST_RESULT = None

_nc_cache = None


def _build_nc():
    nc = bacc.Bacc("TRN2", target_bir_lowering=False, debug=False)
    qT = nc.dram_tensor("qT", [EQ, S], F16, kind="ExternalInput").ap()
    kT = nc.dram_tensor("kT", [EQ, S], F16, kind="ExternalInput").ap()
    vT = nc.dram_tensor("vT", [EV, S], F16, kind="ExternalInput").ap()
    wqT = nc.dram_tensor("wqT", [EQ, F], F16, kind="ExternalInput").ap()
    wkT = nc.dram_tensor("wkT", [EQ, F], F16, kind="ExternalInput").ap()
    wvT = nc.dram_tensor("wvT", [EV, F], F16, kind="ExternalInput").ap()
    woT = nc.dram_tensor("woT", [F, D], F16, kind="ExternalInput").ap()
    cosf = nc.dram_tensor("cosf", [P, S], F16, kind="ExternalInput").ap()
    sinf = nc.dram_tensor("sinf", [P, S], F16, kind="ExternalInput").ap()
    maskA = nc.dram_tensor("maskA", [P, P], F16, kind="ExternalInput").ap()
    out = nc.dram_tensor("out", [S, D], F32, kind="ExternalOutput").ap()

    EXP = mybir.ActivationFunctionType.Exp

    with tile.TileContext(nc) as tc:
        with (
            tc.tile_pool(name="consts", bufs=1) as consts,
            tc.tile_pool(name="persist", bufs=1) as persist,
            tc.tile_pool(name="insb", bufs=2) as insb_pool,
            tc.tile_pool(name="vsb", bufs=2) as vsb_pool,
            tc.tile_pool(name="rope", bufs=2) as rope_pool,
            tc.tile_pool(name="et", bufs=3) as et_pool,
            tc.tile_pool(name="norm", bufs=2) as norm_pool,
            tc.tile_pool(name="denp", bufs=1) as den_pool,
            tc.tile_pool(name="attnc", bufs=2) as attnc_pool,
            tc.tile_pool(name="outsb", bufs=2) as out_pool,
            tc.tile_pool(name="projps", bufs=2, space="PSUM") as proj_ps,
            tc.tile_pool(name="scps", bufs=2, space="PSUM") as sc_ps,
            tc.tile_pool(name="pops", bufs=2, space="PSUM") as po_ps,
        ):
            # ---- persistent activations
            qpT = persist.tile([P, 4, S], F16, tag="qpT")
            kpT = persist.tile([P, 4, S], F16, tag="kpT")
            vpa = persist.tile([P, 16, 8, 65], F16, tag="vpa")
            nc.vector.memset(vpa[:, :, :, 64:65], 1.0)  # softmax-denominator ones

            # ---- weights + tables, loaded once
            wq_t = consts.tile([P, 16, F], F16, tag="wq")
            wk_t = consts.tile([P, 16, F], F16, tag="wk")
            wv_t = consts.tile([P, 8, F], F16, tag="wv")
            wo_t = consts.tile([P, 4, D], F16, tag="wo")
            cos_t = consts.tile([P, S], F16, tag="cos")
            sin_t = consts.tile([P, S], F16, tag="sin")
            mask_t = consts.tile([P, P], F16, tag="mask")
            # spread the startup loads across all four DMA-issue queues
            wq_dma = [nc.sync, nc.scalar]
            wk_dma = [nc.gpsimd, nc.sync]
            for e in range(16):
                wq_dma[e % 2].dma_start(wq_t[:, e, :], wqT[e * P:(e + 1) * P, :])
                wk_dma[e % 2].dma_start(wk_t[:, e, :], wkT[e * P:(e + 1) * P, :])
            for e in range(8):
                wq_dma[e % 2].dma_start(wv_t[:, e, :], wvT[e * P:(e + 1) * P, :])
            for fi in range(4):
                nc.gpsimd.dma_start(wo_t[:, fi, :], woT[fi * P:(fi + 1) * P, :])
            nc.scalar.dma_start(cos_t[:], cosf)
            nc.scalar.dma_start(sin_t[:], sinf)
            nc.gpsimd.dma_start(mask_t[:], maskA)

            def qk_chunk(c):
                """Project q and k for s-chunk c (fp16 matmuls) + rope."""
                ssl = slice(c * 512, (c + 1) * 512)
                for xT, w_t, dstT, nm, dmae in ((qT, wq_t, qpT, "q", nc.sync),
                                                (kT, wk_t, kpT, "k", nc.scalar)):
                    xsb = insb_pool.tile([P, 16, 512], F16, tag="insb",
                                         name=f"{nm}sb{c}")
                    for e in range(16):
                        dmae.dma_start(xsb[:, e, :], xT[e * P:(e + 1) * P, ssl])
                    for ci in range(4):
                        ps = proj_ps.tile([P, 512], F32, tag="proj",
                                          name=f"ps{nm}{c}_{ci}")
                        for e in range(16):
                            nc.tensor.matmul(ps[:], w_t[:, e, ci * P:(ci + 1) * P],
                                             xsb[:, e, :],
                                             start=(e == 0), stop=(e == 15))
                        # rope: out = x*cos + pairswap(x*sin')
                        a16 = rope_pool.tile([P, 512], F16, tag="ropeA")
                        nc.scalar.copy(a16[:], ps[:])
                        cm = rope_pool.tile([P, 512], F16, tag="ropeC")
                        nc.gpsimd.tensor_mul(cm[:], a16[:], cos_t[:, ssl])
                        sm = rope_pool.tile([P, 512], F16, tag="ropeS")
                        nc.gpsimd.tensor_mul(sm[:], a16[:], sin_t[:, ssl])
                        sm2 = rope_pool.tile([P, 512], F16, tag="ropeS2")
                        nc.vector.stream_shuffle(sm2[:], sm[:], SWAP_MASK)
                        nc.vector.tensor_add(dstT[:, ci, ssl], cm[:], sm2[:])

            def v_chunk(stq):
                """Project v for s-chunk stq into vpa (s on partitions)."""
                vsb = vsb_pool.tile([P, 8, 512], F16, tag="vsb", name=f"vsb{stq}")
                for e in range(8):
                    nc.gpsimd.dma_start(vsb[:, e, :],
                                        vT[e * P:(e + 1) * P,
                                           stq * 512:(stq + 1) * 512])
                for j in range(4):
                    ps = proj_ps.tile([P, 512], F32, tag="proj",
                                      name=f"psv{stq}_{j}")
                    for e in range(8):
                        nc.tensor.matmul(ps[:], vsb[:, e, j * P:(j + 1) * P],
                                         wv_t[:, e, :],
                                         start=(e == 0), stop=(e == 7))
                    st = stq * 4 + j
                    nc.scalar.copy(vpa[:, st, :, 0:64],
                                   ps[:].rearrange("p (h d) -> p h d", h=8))

            def attn_chunk(c, attn_c):
                """Causal attention for query chunk c (scoresT layout)."""
                nt = 4 * (c + 1)
                for hp in range(4):
                    po_a = po_ps.tile([P, 512], F32, tag="po", name=f"poa{c}_{hp}")
                    po_b = po_ps.tile([P, 512], F32, tag="po", name=f"pob{c}_{hp}")
                    for t in range(nt):
                        tsl = slice(t * P, (t + 1) * P)
                        rr = P * (t - 4 * c) if t >= 4 * c else 0
                        qsl = slice(c * 512 + rr, (c + 1) * 512)
                        ps_s = sc_ps.tile([P, 2, 512], F32, tag="sc",
                                          name=f"scs{c}_{hp}_{t}")
                        nc.tensor.matmul(ps_s[:, 0, rr:512], kpT[0:64, hp, tsl],
                                         qpT[0:64, hp, qsl], start=True, stop=True)
                        nc.tensor.matmul(ps_s[:, 1, rr:512], kpT[64:128, hp, tsl],
                                         qpT[64:128, hp, qsl], start=True, stop=True)
                        et = et_pool.tile([P, 2, 512], F16, tag="et")
                        nc.scalar.activation(et[:, :, rr:512], ps_s[:, :, rr:512],
                                             EXP, scale=SCALE)
                        if t >= 4 * c:
                            # zero the above-diagonal triangle of this block
                            nc.vector.tensor_mul(
                                et[:, :, rr:rr + P], et[:, :, rr:rr + P],
                                mask_t[:, None, :].to_broadcast((P, 2, P)))
                        nc.tensor.matmul(po_a[0:65, rr:512], vpa[:, t, 2 * hp, :],
                                         et[:, 0, rr:512],
                                         start=(t == 0), stop=(t == nt - 1))
                        nc.tensor.matmul(po_b[0:65, rr:512], vpa[:, t, 2 * hp + 1, :],
                                         et[:, 1, rr:512],
                                         start=(t == 0), stop=(t == nt - 1))
                    # normalize: attn = po[0:64] * (1 / po[64]) per head.
                    # All DVE/gpsimd ops keep inputs at base partition 0
                    # (cross-base inputs miscompile; out-offset is fine).
                    denA = den_pool.tile([1, 512], F32, tag="denA")
                    nc.vector.tensor_copy(denA[:], po_a[64:65, :])
                    denB = den_pool.tile([1, 512], F32, tag="denB")
                    nc.vector.tensor_copy(denB[:], po_b[64:65, :])
                    rcfa = den_pool.tile([1, 512], F32, tag="rcfa")
                    nc.vector.reciprocal_approx_fast(out=rcfa[:], in_=denA[:])
                    rcfb = den_pool.tile([1, 512], F32, tag="rcfb")
                    nc.vector.reciprocal_approx_fast(out=rcfb[:], in_=denB[:])
                    bcA = norm_pool.tile([64, 512], F32, tag="bcA")
                    nc.gpsimd.partition_broadcast(bcA[:], rcfa[:])
                    bcB = norm_pool.tile([64, 512], F32, tag="bcB")
                    nc.gpsimd.partition_broadcast(bcB[:], rcfb[:])
                    posbA = norm_pool.tile([64, 512], F16, tag="posbA")
                    nc.vector.tensor_copy(posbA[:], po_a[0:64, :])
                    posbB = norm_pool.tile([64, 512], F16, tag="posbB")
                    nc.vector.tensor_copy(posbB[:], po_b[0:64, :])
                    nc.vector.tensor_mul(attn_c[0:64, hp, :], posbA[:], bcA[:])
                    nc.vector.tensor_mul(attn_c[64:128, hp, :], posbB[:], bcB[:])

            def outproj(c, attn_c):
                for j in range(4):
                    pw = [po_ps.tile([P, 512], F32, tag="po", name=f"pw{c}_{j}_{i}")
                          for i in range(2)]
                    for ci in range(4):
                        for oc in range(2):
                            nc.tensor.matmul(pw[oc][:],
                                             attn_c[:, ci, j * P:(j + 1) * P],
                                             wo_t[:, ci, oc * 512:(oc + 1) * 512],
                                             start=(ci == 0), stop=(ci == 3))
                    row = (4 * c + j) * P
                    for oc in range(2):
                        ot = out_pool.tile([P, 512], F32, tag="ot")
                        nc.vector.tensor_copy(ot[:], pw[oc][:])
                        nc.gpsimd.dma_start(out[row:row + P, oc * 512:(oc + 1) * 512],
                                            ot[:])

            # ---- program: chunk 0 projections, then per chunk: attention,
            # next-chunk projections (scheduler filler for the PE), out-proj.
            qk_chunk(0)
            v_chunk(0)
            for c in range(4):
                attn_c = attnc_pool.tile([P, 4, 512], F16, tag="attn",
                                         name=f"attn{c}")
                attn_chunk(c, attn_c)
                if c < 3:
                    qk_chunk(c + 1)
                    v_chunk(c + 1)
                outproj(c, attn_c)
    nc.compile()
    return nc


def _tables():
    inv = (1.0 / (ROPE_BASE ** (np.arange(0, Dh, 2, dtype=np.float32) / Dh))
           ).astype(np.float32)                      # [32]
    pos = np.arange(S, dtype=np.float32)
    ang = pos[:, None] * inv[None, :]                # [S, 32]
    cos = np.cos(ang).astype(np.float32)
    sin = np.sin(ang).astype(np.float32)
    d = np.arange(P) % Dh
    i = d // 2
    cosf = np.ascontiguousarray(cos[:, i].T).astype(np.float16)   # [128, S]
    sgn = np.where(d % 2 == 0, 1.0, -1.0).astype(np.float32)
    sinf = np.ascontiguousarray(sin[:, i].T * sgn[:, None]).astype(np.float16)

    p = np.arange(P)
    j = np.arange(P)
    maskA = np.where(p[:, None] <= j[None, :], 1.0, 0.0).astype(np.float16)
    return cosf, sinf, maskA


def kernel(q, k, v, W_q, W_k, W_v, W_o):
    global _nc_cache, LAST_RESULT
    if _nc_cache is None:
        _nc_cache = _build_nc()
    nc = _nc_cache

    cosf, sinf, maskA = _tables()
    q = np.asarray(q, dtype=np.float32)
    k = np.asarray(k, dtype=np.float32)
    v = np.asarray(v, dtype=np.float32)
    W_q = np.asarray(W_q, dtype=np.float32)
    W_k = np.asarray(W_k, dtype=np.float32)
    W_v = np.asarray(W_v, dtype=np.float32)
    W_o = np.asarray(W_o, dtype=np.float32)

    in_maps = []
    for b in range(B):
        qTb = np.ascontiguousarray(q[b].T).astype(np.float16)
        kTb = np.ascontiguousarray(k[b].T).astype(np.float16)
        vTb = np.ascontiguousarray(v[b].T).astype(np.float16)
        for g in range(2):
            fs = slice(g * F, (g + 1) * F)
            in_maps.append({
                "qT": qTb, "kT": kTb, "vT": vTb,
                "wqT": np.ascontiguousarray(W_q[fs, :].T).astype(np.float16),
                "wkT": np.ascontiguousarray(W_k[fs, :].T).astype(np.float16),
                "wvT": np.ascontiguousarray(W_v[fs, :].T).astype(np.float16),
                "woT": np.ascontiguousarray(W_o[:, fs].T).astype(np.float16),
                "cosf": cosf, "sinf": sinf, "maskA": maskA,
            })

    res = bass_utils.run_bass_kernel_spmd(
        nc, in_maps, core_ids=list(range(N_CORES)), trace=KERNEL_TRACE)
    LAST_RESULT = res

    final = np.empty((B, S, D), dtype=np.float32)
    for b in range(B):
        final[b] = res.results[2 * b]["out"] + res.results[2 * b + 1]["out"]
    return final


# revision 19
# speedup vs baseline: 1.3521x; 1.0403x over previous
"""TRN2 Bass kernel for nn_Attention_23493471109551 (v2, fp16).

Full attention layer: QKV projections + interleaved RoPE + causal softmax
attention + output projection, for B=4, S=2048, D=1024, H=16, Dh=64, fp32 I/O.

Sharding: 8 cores = 4 batches x 2 head-groups (8 heads each).  Each core
computes its batch/head-group's attention and a partial output projection
(W_o row-block); host sums the two partials per batch.

v2 changes vs v1 (fp32r baseline, 707us):
  - all matmul operands fp16 (err 3.4e-4 vs 2e-2 budget): halves HBM traffic,
    removes the fp32r n<256 4x penalty, faster ldweights.
  - weights hoisted to SBUF once (v1 reloaded W_q/W_k 4x: 25MB extra DMA).
  - causal mask applied as a 0/1 fp16 multiply on the exp output (SBUF)
    instead of -1e30 add on the score PSUM: cheaper and shortens the
    PSUM critical path.
  - softmax denominator reciprocal via reciprocal_approx_fast (v1's
    nc.vector.reciprocal was 3.3us per call, 106us total DVE).
  - per-chunk emission interleave: projection chunk c+1 is emitted between
    attention chunk c and its output projection, so the tile scheduler can
    fill the (Activation-bound) attention phase with projection matmuls and
    keep the PE continuously busy at its top p-state.

Layout (per core):
  qpT/kpT: [dh-on-partitions (2 heads x 64), hp, S] fp16
  scoresT [sk, sq] in PSUM; exp'd (scale fused) to fp16 et; PV feeds from et
  directly; denominator = ones-column appended to V (row 64 of the PV psum);
  normalization = approx-reciprocal + gpsimd partition_broadcast + fp16 mul.
"""
import math
import numpy as np

import concourse.bass as bass
import concourse.tile as tile
import concourse.mybir as mybir
from concourse import bacc, bass_utils

# problem constants
B, S, D = 4, 2048, 1024
H, Dh = 16, 64
EQ, EV = 2048, 1024          # q/k and v input feature dims
F = 512                      # features per core (8 heads x 64)
P = 128
N_CORES = 8
SCALE = 1.0 / math.sqrt(D)   # 1/32
ROPE_BASE = 10000.0
SWAP_MASK = [i ^ 1 for i in range(32)]

F16 = mybir.dt.float16
F32 = mybir.dt.float32

# test hooks (harness ignores these)
KERNEL_TRACE = False
LAST_RESULT = None

_nc_cache = None


def _build_nc():
    nc = bacc.Bacc("TRN2", target_bir_lowering=False, debug=False)
    qT = nc.dram_tensor("qT", [EQ, S], F16, kind="ExternalInput").ap()
    kT = nc.dram_tensor("kT", [EQ, S], F16, kind="ExternalInput").ap()
    vT = nc.dram_tensor("vT", [EV, S], F16, kind="ExternalInput").ap()
    wqT = nc.dram_tensor("wqT", [EQ, F], F16, kind="ExternalInput").ap()
    wkT = nc.dram_tensor("wkT", [EQ, F], F16, kind="ExternalInput").ap()
    wvT = nc.dram_tensor("wvT", [EV, F], F16, kind="ExternalInput").ap()
    woT = nc.dram_tensor("woT", [F, D], F16, kind="ExternalInput").ap()
    cosf = nc.dram_tensor("cosf", [P, S], F16, kind="ExternalInput").ap()
    sinf = nc.dram_tensor("sinf", [P, S], F16, kind="ExternalInput").ap()
    maskA = nc.dram_tensor("maskA", [P, P], F16, kind="ExternalInput").ap()
    out = nc.dram_tensor("out", [S, D], F32, kind="ExternalOutput").ap()

    EXP = mybir.ActivationFunctionType.Exp

    with tile.TileContext(nc) as tc:
        with (
            tc.tile_pool(name="consts", bufs=1) as consts,
            tc.tile_pool(name="persist", bufs=1) as persist,
            tc.tile_pool(name="insb", bufs=2) as insb_pool,
            tc.tile_pool(name="vsb", bufs=2) as vsb_pool,
            tc.tile_pool(name="rope", bufs=2) as rope_pool,
            tc.tile_pool(name="et", bufs=3) as et_pool,
            tc.tile_pool(name="norm", bufs=2) as norm_pool,
            tc.tile_pool(name="denp", bufs=1) as den_pool,
            tc.tile_pool(name="attnc", bufs=2) as attnc_pool,
            tc.tile_pool(name="outsb", bufs=2) as out_pool,
            tc.tile_pool(name="projps", bufs=2, space="PSUM") as proj_ps,
            tc.tile_pool(name="scps", bufs=2, space="PSUM") as sc_ps,
            tc.tile_pool(name="pops", bufs=2, space="PSUM") as po_ps,
        ):
            # ---- persistent activations
            qpT = persist.tile([P, 4, S], F16, tag="qpT")
            kpT = persist.tile([P, 4, S], F16, tag="kpT")
            vpa = persist.tile([P, 16, 8, 65], F16, tag="vpa")
            nc.vector.memset(vpa[:, :, :, 64:65], 1.0)  # softmax-denominator ones

            # ---- weights + tables, loaded once
            wq_t = consts.tile([P, 16, F], F16, tag="wq")
            wk_t = consts.tile([P, 16, F], F16, tag="wk")
            wv_t = consts.tile([P, 8, F], F16, tag="wv")
            wo_t = consts.tile([P, 4, D], F16, tag="wo")
            cos_t = consts.tile([P, S], F16, tag="cos")
            sin_t = consts.tile([P, S], F16, tag="sin")
            mask_t = consts.tile([P, P], F16, tag="mask")
            # 3D views of the HBM inputs: [p, e, s]
            qTr = qT.rearrange("(e p) s -> p e s", p=P)
            kTr = kT.rearrange("(e p) s -> p e s", p=P)
            vTr = vT.rearrange("(e p) s -> p e s", p=P)
            wqTr = wqT.rearrange("(e p) f -> p e f", p=P)
            wkTr = wkT.rearrange("(e p) f -> p e f", p=P)
            wvTr = wvT.rearrange("(e p) f -> p e f", p=P)
            woTr = woT.rearrange("(e p) f -> p e f", p=P)

            def bulk(dmae, dst, src, n_e, split=4):
                step = n_e // split
                for i in range(0, n_e, step):
                    dmae.dma_start(dst[:, i:i + step], src[:, i:i + step])

            def load_qk(c):
                ssl = slice(c * 512, (c + 1) * 512)
                qsb = insb_pool.tile([P, 16, 512], F16, tag="insb", name=f"qsb{c}")
                bulk(nc.sync, qsb, qTr[:, :, ssl], 16)
                ksb = insb_pool.tile([P, 16, 512], F16, tag="insb", name=f"ksb{c}")
                bulk(nc.scalar, ksb, kTr[:, :, ssl], 16)
                return qsb, ksb

            def load_v(stq):
                vsb = vsb_pool.tile([P, 8, 512], F16, tag="vsb", name=f"vsb{stq}")
                bulk(nc.gpsimd, vsb, vTr[:, :, stq * 512:(stq + 1) * 512], 8, 2)
                return vsb

            # startup loads, ordered by chunk-0 criticality per queue
            bulk(nc.gpsimd, wq_t, wqTr, 16)
            nc.scalar.dma_start(cos_t[:], cosf)
            nc.scalar.dma_start(sin_t[:], sinf)
            qsb0, ksb0 = load_qk(0)
            bulk(nc.scalar, wk_t, wkTr, 16)
            bulk(nc.gpsimd, wv_t, wvTr, 8, 2)
            vsb0 = load_v(0)
            bulk(nc.gpsimd, wo_t, woTr, 4, 2)
            nc.gpsimd.dma_start(mask_t[:], maskA)

            def proj_qk(c, qsb, ksb):
                """Project q and k for s-chunk c (fp16 matmuls) + rope."""
                ssl = slice(c * 512, (c + 1) * 512)
                for xsb, w_t, dstT, nm in ((qsb, wq_t, qpT, "q"),
                                           (ksb, wk_t, kpT, "k")):
                    for ci in range(4):
                        ps = proj_ps.tile([P, 512], F32, tag="proj",
                                          name=f"ps{nm}{c}_{ci}")
                        for e in range(16):
                            nc.tensor.matmul(ps[:], w_t[:, e, ci * P:(ci + 1) * P],
                                             xsb[:, e, :],
                                             start=(e == 0), stop=(e == 15))
                        # rope: out = x*cos + pairswap(x*sin')
                        a16 = rope_pool.tile([P, 512], F16, tag="ropeA")
                        nc.scalar.copy(a16[:], ps[:])
                        cm = rope_pool.tile([P, 512], F16, tag="ropeC")
                        nc.gpsimd.tensor_mul(cm[:], a16[:], cos_t[:, ssl])
                        sm = rope_pool.tile([P, 512], F16, tag="ropeS")
                        nc.gpsimd.tensor_mul(sm[:], a16[:], sin_t[:, ssl])
                        sm2 = rope_pool.tile([P, 512], F16, tag="ropeS2")
                        nc.vector.stream_shuffle(sm2[:], sm[:], SWAP_MASK)
                        nc.vector.tensor_add(dstT[:, ci, ssl], cm[:], sm2[:])

            def proj_v(stq, vsb):
                """Project v for s-chunk stq into vpa (s on partitions)."""
                for j in range(4):
                    ps = proj_ps.tile([P, 512], F32, tag="proj",
                                      name=f"psv{stq}_{j}")
                    for e in range(8):
                        nc.tensor.matmul(ps[:], vsb[:, e, j * P:(j + 1) * P],
                                         wv_t[:, e, :],
                                         start=(e == 0), stop=(e == 7))
                    st = stq * 4 + j
                    nc.scalar.copy(vpa[:, st, :, 0:64],
                                   ps[:].rearrange("p (h d) -> p h d", h=8))

            def attn_chunk(c, attn_c):
                """Causal attention for query chunk c (scoresT layout)."""
                nt = 4 * (c + 1)
                for hp in range(4):
                    po_a = po_ps.tile([P, 512], F32, tag="po", name=f"poa{c}_{hp}")
                    po_b = po_ps.tile([P, 512], F32, tag="po", name=f"pob{c}_{hp}")
                    for t in range(nt):
                        tsl = slice(t * P, (t + 1) * P)
                        rr = P * (t - 4 * c) if t >= 4 * c else 0
                        qsl = slice(c * 512 + rr, (c + 1) * 512)
                        ps_s = sc_ps.tile([P, 2, 512], F32, tag="sc",
                                          name=f"scs{c}_{hp}_{t}")
                        nc.tensor.matmul(ps_s[:, 0, rr:512], kpT[0:64, hp, tsl],
                                         qpT[0:64, hp, qsl], start=True, stop=True)
                        nc.tensor.matmul(ps_s[:, 1, rr:512], kpT[64:128, hp, tsl],
                                         qpT[64:128, hp, qsl], start=True, stop=True)
                        et = et_pool.tile([P, 2, 512], F16, tag="et")
                        nc.scalar.activation(et[:, :, rr:512], ps_s[:, :, rr:512],
                                             EXP, scale=SCALE)
                        if t >= 4 * c:
                            # zero the above-diagonal triangle of this block
                            nc.vector.tensor_mul(
                                et[:, :, rr:rr + P], et[:, :, rr:rr + P],
                                mask_t[:, None, :].to_broadcast((P, 2, P)))
                        nc.tensor.matmul(po_a[0:65, rr:512], vpa[:, t, 2 * hp, :],
                                         et[:, 0, rr:512],
                                         start=(t == 0), stop=(t == nt - 1))
                        nc.tensor.matmul(po_b[0:65, rr:512], vpa[:, t, 2 * hp + 1, :],
                                         et[:, 1, rr:512],
                                         start=(t == 0), stop=(t == nt - 1))
                    # normalize: attn = po[0:64] * (1 / po[64]) per head.
                    # All DVE/gpsimd ops keep inputs at base partition 0
                    # (cross-base inputs miscompile; out-offset is fine).
                    denA = den_pool.tile([1, 512], F32, tag="denA")
                    nc.vector.tensor_copy(denA[:], po_a[64:65, :])
                    denB = den_pool.tile([1, 512], F32, tag="denB")
                    nc.vector.tensor_copy(denB[:], po_b[64:65, :])
                    rcfa = den_pool.tile([1, 512], F32, tag="rcfa")
                    nc.vector.reciprocal_approx_fast(out=rcfa[:], in_=denA[:])
                    rcfb = den_pool.tile([1, 512], F32, tag="rcfb")
                    nc.vector.reciprocal_approx_fast(out=rcfb[:], in_=denB[:])
                    bcA = norm_pool.tile([64, 512], F32, tag="bcA")
                    nc.gpsimd.partition_broadcast(bcA[:], rcfa[:])
                    bcB = norm_pool.tile([64, 512], F32, tag="bcB")
                    nc.gpsimd.partition_broadcast(bcB[:], rcfb[:])
                    posbA = norm_pool.tile([64, 512], F16, tag="posbA")
                    nc.vector.tensor_copy(posbA[:], po_a[0:64, :])
                    posbB = norm_pool.tile([64, 512], F16, tag="posbB")
                    nc.vector.tensor_copy(posbB[:], po_b[0:64, :])
                    nc.vector.tensor_mul(attn_c[0:64, hp, :], posbA[:], bcA[:])
                    nc.vector.tensor_mul(attn_c[64:128, hp, :], posbB[:], bcB[:])

            def outproj(c, attn_c):
                for j in range(4):
                    pw = [po_ps.tile([P, 512], F32, tag="po", name=f"pw{c}_{j}_{i}")
                          for i in range(2)]
                    for ci in range(4):
                        for oc in range(2):
                            nc.tensor.matmul(pw[oc][:],
                                             attn_c[:, ci, j * P:(j + 1) * P],
                                             wo_t[:, ci, oc * 512:(oc + 1) * 512],
                                             start=(ci == 0), stop=(ci == 3))
                    row = (4 * c + j) * P
                    for oc in range(2):
                        ot = out_pool.tile([P, 512], F32, tag="ot")
                        nc.vector.tensor_copy(ot[:], pw[oc][:])
                        nc.gpsimd.dma_start(out[row:row + P, oc * 512:(oc + 1) * 512],
                                            ot[:])

            # ---- program: chunk 0 projections, then per chunk: attention,
            # next-chunk projections (scheduler filler for the PE), out-proj.
            proj_qk(0, qsb0, ksb0)
            proj_v(0, vsb0)
            for c in range(4):
                attn_c = attnc_pool.tile([P, 4, 512], F16, tag="attn",
                                         name=f"attn{c}")
                attn_chunk(c, attn_c)
                if c < 3:
                    qsb, ksb = load_qk(c + 1)
                    vsb = load_v(c + 1)
                    proj_qk(c + 1, qsb, ksb)
                    proj_v(c + 1, vsb)
                outproj(c, attn_c)
    nc.compile()
    return nc


def _tables():
    inv = (1.0 / (ROPE_BASE ** (np.arange(0, Dh, 2, dtype=np.float32) / Dh))
           ).astype(np.float32)                      # [32]
    pos = np.arange(S, dtype=np.float32)
    ang = pos[:, None] * inv[None, :]                # [S, 32]
    cos = np.cos(ang).astype(np.float32)
    sin = np.sin(ang).astype(np.float32)
    d = np.arange(P) % Dh
    i = d // 2
    cosf = np.ascontiguousarray(cos[:, i].T).astype(np.float16)   # [128, S]
    sgn = np.where(d % 2 == 0, 1.0, -1.0).astype(np.float32)
    sinf = np.ascontiguousarray(sin[:, i].T * sgn[:, None]).astype(np.float16)

    p = np.arange(P)
    j = np.arange(P)
    maskA = np.where(p[:, None] <= j[None, :], 1.0, 0.0).astype(np.float16)
    return cosf, sinf, maskA


def kernel(q, k, v, W_q, W_k, W_v, W_o):
    global _nc_cache, LAST_RESULT
    if _nc_cache is None:
        _nc_cache = _build_nc()
    nc = _nc_cache

    cosf, sinf, maskA = _tables()
    q = np.asarray(q, dtype=np.float32)
    k = np.asarray(k, dtype=np.float32)
    v = np.asarray(v, dtype=np.float32)
    W_q = np.asarray(W_q, dtype=np.float32)
    W_k = np.asarray(W_k, dtype=np.float32)
    W_v = np.asarray(W_v, dtype=np.float32)
    W_o = np.asarray(W_o, dtype=np.float32)

    in_maps = []
    for b in range(B):
        qTb = np.ascontiguousarray(q[b].T).astype(np.float16)
        kTb = np.ascontiguousarray(k[b].T).astype(np.float16)
        vTb = np.ascontiguousarray(v[b].T).astype(np.float16)
        for g in range(2):
            fs = slice(g * F, (g + 1) * F)
            in_maps.append({
                "qT": qTb, "kT": kTb, "vT": vTb,
                "wqT": np.ascontiguousarray(W_q[fs, :].T).astype(np.float16),
                "wkT": np.ascontiguousarray(W_k[fs, :].T).astype(np.float16),
                "wvT": np.ascontiguousarray(W_v[fs, :].T).astype(np.float16),
                "woT": np.ascontiguousarray(W_o[:, fs].T).astype(np.float16),
                "cosf": cosf, "sinf": sinf, "maskA": maskA,
            })

    res = bass_utils.run_bass_kernel_spmd(
        nc, in_maps, core_ids=list(range(N_CORES)), trace=KERNEL_TRACE)
    LAST_RESULT = res

    final = np.empty((B, S, D), dtype=np.float32)
    for b in range(B):
        final[b] = res.results[2 * b]["out"] + res.results[2 * b + 1]["out"]
    return final


# revision 22
# speedup vs baseline: 1.7541x; 1.2973x over previous
"""TRN2 Bass kernel for nn_Attention_23493471109551 (v2, fp16).

Full attention layer: QKV projections + interleaved RoPE + causal softmax
attention + output projection, for B=4, S=2048, D=1024, H=16, Dh=64, fp32 I/O.

Sharding: 8 cores = 4 batches x 2 head-groups (8 heads each).  Each core
computes its batch/head-group's attention and a partial output projection
(W_o row-block); host sums the two partials per batch.

v2 changes vs v1 (fp32r baseline, 707us):
  - all matmul operands fp16 (err 3.4e-4 vs 2e-2 budget): halves HBM traffic,
    removes the fp32r n<256 4x penalty, faster ldweights.
  - weights hoisted to SBUF once (v1 reloaded W_q/W_k 4x: 25MB extra DMA).
  - causal mask applied as a 0/1 fp16 multiply on the exp output (SBUF)
    instead of -1e30 add on the score PSUM: cheaper and shortens the
    PSUM critical path.
  - softmax denominator reciprocal via reciprocal_approx_fast (v1's
    nc.vector.reciprocal was 3.3us per call, 106us total DVE).
  - per-chunk emission interleave: projection chunk c+1 is emitted between
    attention chunk c and its output projection, so the tile scheduler can
    fill the (Activation-bound) attention phase with projection matmuls and
    keep the PE continuously busy at its top p-state.

Layout (per core):
  qpT/kpT: [dh-on-partitions (2 heads x 64), hp, S] fp16
  scoresT [sk, sq] in PSUM; exp'd (scale fused) to fp16 et; PV feeds from et
  directly; denominator = ones-column appended to V (row 64 of the PV psum);
  normalization = approx-reciprocal + gpsimd partition_broadcast + fp16 mul.
"""
import math
import numpy as np

import concourse.bass as bass
import concourse.tile as tile
import concourse.mybir as mybir
from concourse import bacc, bass_utils

# problem constants
B, S, D = 4, 2048, 1024
H, Dh = 16, 64
EQ, EV = 2048, 1024          # q/k and v input feature dims
F = 512                      # features per core (8 heads x 64)
P = 128
N_CORES = 8
SCALE = 1.0 / math.sqrt(D)   # 1/32
ROPE_BASE = 10000.0
SWAP_MASK = [i ^ 1 for i in range(32)]

F16 = mybir.dt.float16
F32 = mybir.dt.float32

# test hooks (harness ignores these)
KERNEL_TRACE = False
LAST_RESULT = None

_nc_cache = None


def _build_nc():
    nc = bacc.Bacc("TRN2", target_bir_lowering=False, debug=False)
    qT = nc.dram_tensor("qT", [EQ, S], F16, kind="ExternalInput").ap()
    kT = nc.dram_tensor("kT", [EQ, S], F16, kind="ExternalInput").ap()
    vT = nc.dram_tensor("vT", [EV, S], F16, kind="ExternalInput").ap()
    wqT = nc.dram_tensor("wqT", [EQ, F], F16, kind="ExternalInput").ap()
    wkT = nc.dram_tensor("wkT", [EQ, F], F16, kind="ExternalInput").ap()
    wvT = nc.dram_tensor("wvT", [EV, F], F16, kind="ExternalInput").ap()
    woT = nc.dram_tensor("woT", [F, D], F16, kind="ExternalInput").ap()
    cosf = nc.dram_tensor("cosf", [P, S], F16, kind="ExternalInput").ap()
    sinf = nc.dram_tensor("sinf", [P, S], F16, kind="ExternalInput").ap()
    maskA = nc.dram_tensor("maskA", [P, P], F16, kind="ExternalInput").ap()
    out = nc.dram_tensor("out", [S, D], F32, kind="ExternalOutput").ap()

    EXP = mybir.ActivationFunctionType.Exp

    with tile.TileContext(nc) as tc:
        with (
            tc.tile_pool(name="consts", bufs=1) as consts,
            tc.tile_pool(name="persist", bufs=1) as persist,
            tc.tile_pool(name="insb", bufs=2) as insb_pool,
            tc.tile_pool(name="vsb", bufs=2) as vsb_pool,
            tc.tile_pool(name="rope", bufs=2) as rope_pool,
            tc.tile_pool(name="et", bufs=3) as et_pool,
            tc.tile_pool(name="norm", bufs=2) as norm_pool,
            tc.tile_pool(name="denp", bufs=1) as den_pool,
            tc.tile_pool(name="attnc", bufs=2) as attnc_pool,
            tc.tile_pool(name="outsb", bufs=2) as out_pool,
            tc.tile_pool(name="projps", bufs=2, space="PSUM") as proj_ps,
            tc.tile_pool(name="scps", bufs=2, space="PSUM") as sc_ps,
            tc.tile_pool(name="pops", bufs=2, space="PSUM") as po_ps,
        ):
            # ---- persistent activations
            qpT = persist.tile([P, 4, S], F16, tag="qpT")
            kpT = persist.tile([P, 4, S], F16, tag="kpT")
            vpa = persist.tile([P, 16, 8, 65], F16, tag="vpa")
            nc.vector.memset(vpa[:, :, :, 64:65], 1.0)  # softmax-denominator ones

            # ---- weights + tables, loaded once
            wq_t = consts.tile([P, 16, F], F16, tag="wq")
            wk_t = consts.tile([P, 16, F], F16, tag="wk")
            wv_t = consts.tile([P, 8, F], F16, tag="wv")
            wo_t = consts.tile([P, 4, D], F16, tag="wo")
            cos_t = consts.tile([P, S], F16, tag="cos")
            sin_t = consts.tile([P, S], F16, tag="sin")
            mask_t = consts.tile([P, P], F16, tag="mask")
            # 3D views of the HBM inputs: [p, e, s]
            qTr = qT.rearrange("(e p) s -> p e s", p=P)
            kTr = kT.rearrange("(e p) s -> p e s", p=P)
            vTr = vT.rearrange("(e p) s -> p e s", p=P)
            wqTr = wqT.rearrange("(e p) f -> p e f", p=P)
            wkTr = wkT.rearrange("(e p) f -> p e f", p=P)
            wvTr = wvT.rearrange("(e p) f -> p e f", p=P)
            woTr = woT.rearrange("(e p) f -> p e f", p=P)

            def bulk(dmae, dst, src, n_e, split=4):
                step = n_e // split
                for i in range(0, n_e, step):
                    dmae.dma_start(dst[:, i:i + step], src[:, i:i + step])

            def load_qk(c):
                ssl = slice(c * 512, (c + 1) * 512)
                qsb = insb_pool.tile([P, 16, 512], F16, tag="insb", name=f"qsb{c}")
                bulk(nc.sync, qsb, qTr[:, :, ssl], 16)
                ksb = insb_pool.tile([P, 16, 512], F16, tag="insb", name=f"ksb{c}")
                bulk(nc.scalar, ksb, kTr[:, :, ssl], 16)
                return qsb, ksb

            def load_v(stq):
                vsb = vsb_pool.tile([P, 8, 512], F16, tag="vsb", name=f"vsb{stq}")
                bulk(nc.gpsimd, vsb, vTr[:, :, stq * 512:(stq + 1) * 512], 8, 2)
                return vsb

            # startup loads, ordered by chunk-0 criticality per queue
            bulk(nc.gpsimd, wq_t, wqTr, 16)
            nc.scalar.dma_start(cos_t[:], cosf)
            nc.scalar.dma_start(sin_t[:], sinf)
            qsb0, ksb0 = load_qk(0)
            bulk(nc.scalar, wk_t, wkTr, 16)
            bulk(nc.gpsimd, wv_t, wvTr, 8, 2)
            vsb0 = load_v(0)
            bulk(nc.gpsimd, wo_t, woTr, 4, 2)
            nc.gpsimd.dma_start(mask_t[:], maskA)

            def proj_qk(c, qsb, ksb):
                """Project q and k for s-chunk c (fp16 matmuls) + rope."""
                ssl = slice(c * 512, (c + 1) * 512)
                for xsb, w_t, dstT, nm in ((qsb, wq_t, qpT, "q"),
                                           (ksb, wk_t, kpT, "k")):
                    for ci in range(4):
                        ps = proj_ps.tile([P, 512], F32, tag="proj",
                                          name=f"ps{nm}{c}_{ci}")
                        for e in range(16):
                            nc.tensor.matmul(ps[:], w_t[:, e, ci * P:(ci + 1) * P],
                                             xsb[:, e, :],
                                             start=(e == 0), stop=(e == 15))
                        # rope: out = x*cos + pairswap(x*sin')
                        a16 = rope_pool.tile([P, 512], F16, tag="ropeA")
                        nc.scalar.copy(a16[:], ps[:])
                        cm = rope_pool.tile([P, 512], F16, tag="ropeC")
                        nc.vector.tensor_mul(cm[:], a16[:], cos_t[:, ssl])
                        sm = rope_pool.tile([P, 512], F16, tag="ropeS")
                        nc.vector.tensor_mul(sm[:], a16[:], sin_t[:, ssl])
                        sm2 = rope_pool.tile([P, 512], F16, tag="ropeS2")
                        nc.vector.stream_shuffle(sm2[:], sm[:], SWAP_MASK)
                        nc.vector.tensor_add(dstT[:, ci, ssl], cm[:], sm2[:])

            def proj_v(stq, vsb):
                """Project v for s-chunk stq into vpa (s on partitions)."""
                for j in range(4):
                    ps = proj_ps.tile([P, 512], F32, tag="proj",
                                      name=f"psv{stq}_{j}")
                    for e in range(8):
                        nc.tensor.matmul(ps[:], vsb[:, e, j * P:(j + 1) * P],
                                         wv_t[:, e, :],
                                         start=(e == 0), stop=(e == 7))
                    st = stq * 4 + j
                    nc.scalar.copy(vpa[:, st, :, 0:64],
                                   ps[:].rearrange("p (h d) -> p h d", h=8))

            def attn_chunk(c, attn_c):
                """Causal attention for query chunk c (scoresT layout)."""
                nt = 4 * (c + 1)
                for hp in range(4):
                    po_a = po_ps.tile([P, 512], F32, tag="po", name=f"poa{c}_{hp}")
                    po_b = po_ps.tile([P, 512], F32, tag="po", name=f"pob{c}_{hp}")
                    for t in range(nt):
                        tsl = slice(t * P, (t + 1) * P)
                        rr = P * (t - 4 * c) if t >= 4 * c else 0
                        qsl = slice(c * 512 + rr, (c + 1) * 512)
                        ps_s = sc_ps.tile([P, 2, 512], F32, tag="sc",
                                          name=f"scs{c}_{hp}_{t}")
                        nc.tensor.matmul(ps_s[:, 0, rr:512], kpT[0:64, hp, tsl],
                                         qpT[0:64, hp, qsl], start=True, stop=True)
                        nc.tensor.matmul(ps_s[:, 1, rr:512], kpT[64:128, hp, tsl],
                                         qpT[64:128, hp, qsl], start=True, stop=True)
                        et = et_pool.tile([P, 2, 512], F16, tag="et")
                        nc.scalar.activation(et[:, :, rr:512], ps_s[:, :, rr:512],
                                             EXP, scale=SCALE)
                        if t >= 4 * c:
                            # zero the above-diagonal triangle of this block
                            nc.vector.tensor_mul(
                                et[:, :, rr:rr + P], et[:, :, rr:rr + P],
                                mask_t[:, None, :].to_broadcast((P, 2, P)))
                        nc.tensor.matmul(po_a[0:65, rr:512], vpa[:, t, 2 * hp, :],
                                         et[:, 0, rr:512],
                                         start=(t == 0), stop=(t == nt - 1))
                        nc.tensor.matmul(po_b[0:65, rr:512], vpa[:, t, 2 * hp + 1, :],
                                         et[:, 1, rr:512],
                                         start=(t == 0), stop=(t == nt - 1))
                    # normalize: attn = po[0:64] * (1 / po[64]) per head.
                    # All DVE/gpsimd ops keep inputs at base partition 0
                    # (cross-base inputs miscompile; out-offset is fine).
                    denA = den_pool.tile([1, 512], F32, tag="denA")
                    nc.vector.tensor_copy(denA[:], po_a[64:65, :])
                    denB = den_pool.tile([1, 512], F32, tag="denB")
                    nc.vector.tensor_copy(denB[:], po_b[64:65, :])
                    rcfa = den_pool.tile([1, 512], F32, tag="rcfa")
                    nc.vector.reciprocal_approx_fast(out=rcfa[:], in_=denA[:])
                    rcfb = den_pool.tile([1, 512], F32, tag="rcfb")
                    nc.vector.reciprocal_approx_fast(out=rcfb[:], in_=denB[:])
                    bcA = norm_pool.tile([64, 512], F32, tag="bcA")
                    nc.gpsimd.partition_broadcast(bcA[:], rcfa[:])
                    bcB = norm_pool.tile([64, 512], F32, tag="bcB")
                    nc.gpsimd.partition_broadcast(bcB[:], rcfb[:])
                    nc.vector.tensor_mul(attn_c[0:64, hp, :], po_a[0:64, :], bcA[:])
                    nc.vector.tensor_mul(attn_c[64:128, hp, :], po_b[0:64, :], bcB[:])

            def outproj(c, attn_c):
                for j in range(4):
                    pw = [po_ps.tile([P, 512], F32, tag="po", name=f"pw{c}_{j}_{i}")
                          for i in range(2)]
                    for ci in range(4):
                        for oc in range(2):
                            nc.tensor.matmul(pw[oc][:],
                                             attn_c[:, ci, j * P:(j + 1) * P],
                                             wo_t[:, ci, oc * 512:(oc + 1) * 512],
                                             start=(ci == 0), stop=(ci == 3))
                    row = (4 * c + j) * P
                    for oc in range(2):
                        ot = out_pool.tile([P, 512], F32, tag="ot")
                        nc.vector.tensor_copy(ot[:], pw[oc][:])
                        nc.sync.dma_start(out[row:row + P, oc * 512:(oc + 1) * 512],
                                          ot[:])

            # ---- program: chunk 0 projections, then per chunk: attention,
            # next-chunk projections (scheduler filler for the PE), out-proj.
            proj_qk(0, qsb0, ksb0)
            proj_v(0, vsb0)
            for c in range(4):
                attn_c = attnc_pool.tile([P, 4, 512], F16, tag="attn",
                                         name=f"attn{c}")
                attn_chunk(c, attn_c)
                if c < 3:
                    qsb, ksb = load_qk(c + 1)
                    vsb = load_v(c + 1)
                    proj_qk(c + 1, qsb, ksb)
                    proj_v(c + 1, vsb)
                outproj(c, attn_c)
    nc.compile()
    return nc


def _tables():
    inv = (1.0 / (ROPE_BASE ** (np.arange(0, Dh, 2, dtype=np.float32) / Dh))
           ).astype(np.float32)                      # [32]
    pos = np.arange(S, dtype=np.float32)
    ang = pos[:, None] * inv[None, :]                # [S, 32]
    cos = np.cos(ang).astype(np.float32)
    sin = np.sin(ang).astype(np.float32)
    d = np.arange(P) % Dh
    i = d // 2
    cosf = np.ascontiguousarray(cos[:, i].T).astype(np.float16)   # [128, S]
    sgn = np.where(d % 2 == 0, 1.0, -1.0).astype(np.float32)
    sinf = np.ascontiguousarray(sin[:, i].T * sgn[:, None]).astype(np.float16)

    p = np.arange(P)
    j = np.arange(P)
    maskA = np.where(p[:, None] <= j[None, :], 1.0, 0.0).astype(np.float16)
    return cosf, sinf, maskA


def kernel(q, k, v, W_q, W_k, W_v, W_o):
    global _nc_cache, LAST_RESULT
    if _nc_cache is None:
        _nc_cache = _build_nc()
    nc = _nc_cache

    cosf, sinf, maskA = _tables()
    q = np.asarray(q, dtype=np.float32)
    k = np.asarray(k, dtype=np.float32)
    v = np.asarray(v, dtype=np.float32)
    W_q = np.asarray(W_q, dtype=np.float32)
    W_k = np.asarray(W_k, dtype=np.float32)
    W_v = np.asarray(W_v, dtype=np.float32)
    W_o = np.asarray(W_o, dtype=np.float32)

    in_maps = []
    for b in range(B):
        qTb = np.ascontiguousarray(q[b].T).astype(np.float16)
        kTb = np.ascontiguousarray(k[b].T).astype(np.float16)
        vTb = np.ascontiguousarray(v[b].T).astype(np.float16)
        for g in range(2):
            fs = slice(g * F, (g + 1) * F)
            in_maps.append({
                "qT": qTb, "kT": kTb, "vT": vTb,
                "wqT": np.ascontiguousarray(W_q[fs, :].T).astype(np.float16),
                "wkT": np.ascontiguousarray(W_k[fs, :].T).astype(np.float16),
                "wvT": np.ascontiguousarray(W_v[fs, :].T).astype(np.float16),
                "woT": np.ascontiguousarray(W_o[:, fs].T).astype(np.float16),
                "cosf": cosf, "sinf": sinf, "maskA": maskA,
            })

    res = bass_utils.run_bass_kernel_spmd(
        nc, in_maps, core_ids=list(range(N_CORES)), trace=KERNEL_TRACE)
    LAST_RESULT = res

    final = np.empty((B, S, D), dtype=np.float32)
    for b in range(B):
        final[b] = res.results[2 * b]["out"] + res.results[2 * b + 1]["out"]
    return final


# revision 24
# speedup vs baseline: 1.8012x; 1.0268x over previous
"""TRN2 Bass kernel for nn_Attention_23493471109551 (v2, fp16).

Full attention layer: QKV projections + interleaved RoPE + causal softmax
attention + output projection, for B=4, S=2048, D=1024, H=16, Dh=64, fp32 I/O.

Sharding: 8 cores = 4 batches x 2 head-groups (8 heads each).  Each core
computes its batch/head-group's attention and a partial output projection
(W_o row-block); host sums the two partials per batch.

v2 changes vs v1 (fp32r baseline, 707us):
  - all matmul operands fp16 (err 3.4e-4 vs 2e-2 budget): halves HBM traffic,
    removes the fp32r n<256 4x penalty, faster ldweights.
  - weights hoisted to SBUF once (v1 reloaded W_q/W_k 4x: 25MB extra DMA).
  - causal mask applied as a 0/1 fp16 multiply on the exp output (SBUF)
    instead of -1e30 add on the score PSUM: cheaper and shortens the
    PSUM critical path.
  - softmax denominator reciprocal via reciprocal_approx_fast (v1's
    nc.vector.reciprocal was 3.3us per call, 106us total DVE).
  - per-chunk emission interleave: projection chunk c+1 is emitted between
    attention chunk c and its output projection, so the tile scheduler can
    fill the (Activation-bound) attention phase with projection matmuls and
    keep the PE continuously busy at its top p-state.

Layout (per core):
  qpT/kpT: [dh-on-partitions (2 heads x 64), hp, S] fp16
  scoresT [sk, sq] in PSUM; exp'd (scale fused) to fp16 et; PV feeds from et
  directly; denominator = ones-column appended to V (row 64 of the PV psum);
  normalization = approx-reciprocal + gpsimd partition_broadcast + fp16 mul.
"""
import math
import numpy as np

import concourse.bass as bass
import concourse.tile as tile
import concourse.mybir as mybir
from concourse import bacc, bass_utils

# problem constants
B, S, D = 4, 2048, 1024
H, Dh = 16, 64
EQ, EV = 2048, 1024          # q/k and v input feature dims
F = 512                      # features per core (8 heads x 64)
P = 128
N_CORES = 8
SCALE = 1.0 / math.sqrt(D)   # 1/32
ROPE_BASE = 10000.0
SWAP_MASK = [i ^ 1 for i in range(32)]

F16 = mybir.dt.float16
F32 = mybir.dt.float32

# test hooks (harness ignores these)
KERNEL_TRACE = False
LAST_RESULT = None

_nc_cache = None


def _build_nc():
    nc = bacc.Bacc("TRN2", target_bir_lowering=False, debug=False)
    qT = nc.dram_tensor("qT", [EQ, S], F16, kind="ExternalInput").ap()
    kT = nc.dram_tensor("kT", [EQ, S], F16, kind="ExternalInput").ap()
    vT = nc.dram_tensor("vT", [EV, S], F16, kind="ExternalInput").ap()
    wqT = nc.dram_tensor("wqT", [EQ, F], F16, kind="ExternalInput").ap()
    wkT = nc.dram_tensor("wkT", [EQ, F], F16, kind="ExternalInput").ap()
    wvT = nc.dram_tensor("wvT", [EV, F], F16, kind="ExternalInput").ap()
    woT = nc.dram_tensor("woT", [F, D], F16, kind="ExternalInput").ap()
    cosf = nc.dram_tensor("cosf", [P, S], F16, kind="ExternalInput").ap()
    sinf = nc.dram_tensor("sinf", [P, S], F16, kind="ExternalInput").ap()
    maskA = nc.dram_tensor("maskA", [P, P], F16, kind="ExternalInput").ap()
    out = nc.dram_tensor("out", [S, D], F32, kind="ExternalOutput").ap()

    EXP = mybir.ActivationFunctionType.Exp

    with tile.TileContext(nc) as tc:
        with (
            tc.tile_pool(name="consts", bufs=1) as consts,
            tc.tile_pool(name="persist", bufs=1) as persist,
            tc.tile_pool(name="insb", bufs=2) as insb_pool,
            tc.tile_pool(name="vsb", bufs=2) as vsb_pool,
            tc.tile_pool(name="rope", bufs=2) as rope_pool,
            tc.tile_pool(name="et", bufs=3) as et_pool,
            tc.tile_pool(name="norm", bufs=2) as norm_pool,
            tc.tile_pool(name="denp", bufs=1) as den_pool,
            tc.tile_pool(name="attnc", bufs=2) as attnc_pool,
            tc.tile_pool(name="outsb", bufs=2) as out_pool,
            tc.tile_pool(name="projps", bufs=2, space="PSUM") as proj_ps,
            tc.tile_pool(name="scps", bufs=2, space="PSUM") as sc_ps,
            tc.tile_pool(name="pops", bufs=2, space="PSUM") as po_ps,
        ):
            # ---- persistent activations
            qpT = persist.tile([P, 4, S], F16, tag="qpT")
            kpT = persist.tile([P, 4, S], F16, tag="kpT")
            vpa = persist.tile([P, 16, 8, 65], F16, tag="vpa")
            nc.vector.memset(vpa[:, :, :, 64:65], 1.0)  # softmax-denominator ones

            # ---- weights + tables, loaded once
            wq_t = consts.tile([P, 16, F], F16, tag="wq")
            wk_t = consts.tile([P, 16, F], F16, tag="wk")
            wv_t = consts.tile([P, 8, F], F16, tag="wv")
            wo_t = consts.tile([P, 4, D], F16, tag="wo")
            cos_t = consts.tile([P, S], F16, tag="cos")
            sin_t = consts.tile([P, S], F16, tag="sin")
            mask_t = consts.tile([P, P], F16, tag="mask")
            # 3D views of the HBM inputs: [p, e, s]
            qTr = qT.rearrange("(e p) s -> p e s", p=P)
            kTr = kT.rearrange("(e p) s -> p e s", p=P)
            vTr = vT.rearrange("(e p) s -> p e s", p=P)
            wqTr = wqT.rearrange("(e p) f -> p e f", p=P)
            wkTr = wkT.rearrange("(e p) f -> p e f", p=P)
            wvTr = wvT.rearrange("(e p) f -> p e f", p=P)
            woTr = woT.rearrange("(e p) f -> p e f", p=P)

            def bulk(dmae, dst, src, n_e, split=4):
                step = n_e // split
                for i in range(0, n_e, step):
                    dmae.dma_start(dst[:, i:i + step], src[:, i:i + step])

            def load_qk(c):
                ssl = slice(c * 512, (c + 1) * 512)
                qsb = insb_pool.tile([P, 16, 512], F16, tag="insb", name=f"qsb{c}")
                bulk(nc.sync, qsb, qTr[:, :, ssl], 16)
                ksb = insb_pool.tile([P, 16, 512], F16, tag="insb", name=f"ksb{c}")
                bulk(nc.scalar, ksb, kTr[:, :, ssl], 16)
                return qsb, ksb

            def load_v(stq):
                vsb = vsb_pool.tile([P, 8, 512], F16, tag="vsb", name=f"vsb{stq}")
                bulk(nc.gpsimd, vsb, vTr[:, :, stq * 512:(stq + 1) * 512], 8, 2)
                return vsb

            # startup loads, ordered by chunk-0 criticality per queue:
            # gpsimd: wq -> wv -> vsb0 ; sync: qsb0 -> wo, mask ;
            # scalar: cos, sin -> ksb0 -> wk
            bulk(nc.gpsimd, wq_t, wqTr, 16)
            nc.scalar.dma_start(cos_t[:], cosf)
            nc.scalar.dma_start(sin_t[:], sinf)
            qsb0, ksb0 = load_qk(0)
            bulk(nc.scalar, wk_t, wkTr, 16)
            bulk(nc.gpsimd, wv_t, wvTr, 8, 2)
            vsb0 = load_v(0)
            bulk(nc.sync, wo_t, woTr, 4, 2)
            nc.sync.dma_start(mask_t[:], maskA)

            def proj_qk(c, qsb, ksb):
                """Project q and k for s-chunk c (fp16 matmuls) + rope."""
                ssl = slice(c * 512, (c + 1) * 512)
                for xsb, w_t, dstT, nm in ((qsb, wq_t, qpT, "q"),
                                           (ksb, wk_t, kpT, "k")):
                    for ci in range(4):
                        ps = proj_ps.tile([P, 512], F32, tag="proj",
                                          name=f"ps{nm}{c}_{ci}")
                        for e in range(16):
                            nc.tensor.matmul(ps[:], w_t[:, e, ci * P:(ci + 1) * P],
                                             xsb[:, e, :],
                                             start=(e == 0), stop=(e == 15))
                        # rope: out = x*cos + pairswap(x*sin')
                        a16 = rope_pool.tile([P, 512], F16, tag="ropeA")
                        nc.scalar.copy(a16[:], ps[:])
                        cm = rope_pool.tile([P, 512], F16, tag="ropeC")
                        nc.vector.tensor_mul(cm[:], a16[:], cos_t[:, ssl])
                        sm = rope_pool.tile([P, 512], F16, tag="ropeS")
                        nc.vector.tensor_mul(sm[:], a16[:], sin_t[:, ssl])
                        sm2 = rope_pool.tile([P, 512], F16, tag="ropeS2")
                        nc.vector.stream_shuffle(sm2[:], sm[:], SWAP_MASK)
                        nc.vector.tensor_add(dstT[:, ci, ssl], cm[:], sm2[:])

            def proj_v(stq, vsb):
                """Project v for s-chunk stq into vpa (s on partitions)."""
                for j in range(4):
                    ps = proj_ps.tile([P, 512], F32, tag="proj",
                                      name=f"psv{stq}_{j}")
                    for e in range(8):
                        nc.tensor.matmul(ps[:], vsb[:, e, j * P:(j + 1) * P],
                                         wv_t[:, e, :],
                                         start=(e == 0), stop=(e == 7))
                    st = stq * 4 + j
                    nc.scalar.copy(vpa[:, st, :, 0:64],
                                   ps[:].rearrange("p (h d) -> p h d", h=8))

            def attn_chunk(c, attn_c):
                """Causal attention for query chunk c (scoresT layout)."""
                nt = 4 * (c + 1)
                for hp in range(4):
                    po_a = po_ps.tile([P, 512], F32, tag="po", name=f"poa{c}_{hp}")
                    po_b = po_ps.tile([P, 512], F32, tag="po", name=f"pob{c}_{hp}")
                    for t in range(nt):
                        tsl = slice(t * P, (t + 1) * P)
                        rr = P * (t - 4 * c) if t >= 4 * c else 0
                        qsl = slice(c * 512 + rr, (c + 1) * 512)
                        ps_s = sc_ps.tile([P, 2, 512], F32, tag="sc",
                                          name=f"scs{c}_{hp}_{t}")
                        nc.tensor.matmul(ps_s[:, 0, rr:512], kpT[0:64, hp, tsl],
                                         qpT[0:64, hp, qsl], start=True, stop=True)
                        nc.tensor.matmul(ps_s[:, 1, rr:512], kpT[64:128, hp, tsl],
                                         qpT[64:128, hp, qsl], start=True, stop=True)
                        et = et_pool.tile([P, 2, 512], F16, tag="et")
                        nc.scalar.activation(et[:, :, rr:512], ps_s[:, :, rr:512],
                                             EXP, scale=SCALE)
                        if t >= 4 * c:
                            # zero the above-diagonal triangle of this block
                            nc.vector.tensor_mul(
                                et[:, :, rr:rr + P], et[:, :, rr:rr + P],
                                mask_t[:, None, :].to_broadcast((P, 2, P)))
                        nc.tensor.matmul(po_a[0:65, rr:512], vpa[:, t, 2 * hp, :],
                                         et[:, 0, rr:512],
                                         start=(t == 0), stop=(t == nt - 1))
                        nc.tensor.matmul(po_b[0:65, rr:512], vpa[:, t, 2 * hp + 1, :],
                                         et[:, 1, rr:512],
                                         start=(t == 0), stop=(t == nt - 1))
                    # normalize: attn = po[0:64] * (1 / po[64]) per head.
                    # All DVE/gpsimd ops keep inputs at base partition 0
                    # (cross-base inputs miscompile; out-offset is fine).
                    denA = den_pool.tile([1, 512], F32, tag="denA")
                    nc.vector.tensor_copy(denA[:], po_a[64:65, :])
                    denB = den_pool.tile([1, 512], F32, tag="denB")
                    nc.vector.tensor_copy(denB[:], po_b[64:65, :])
                    rcfa = den_pool.tile([1, 512], F32, tag="rcfa")
                    nc.vector.reciprocal_approx_fast(out=rcfa[:], in_=denA[:])
                    rcfb = den_pool.tile([1, 512], F32, tag="rcfb")
                    nc.vector.reciprocal_approx_fast(out=rcfb[:], in_=denB[:])
                    # stage po down to fp16 SBUF immediately so the PSUM bank
                    # frees before the (longer) reciprocal/broadcast chain
                    posbA = norm_pool.tile([64, 512], F16, tag="posbA")
                    nc.vector.tensor_copy(posbA[:], po_a[0:64, :])
                    posbB = norm_pool.tile([64, 512], F16, tag="posbB")
                    nc.vector.tensor_copy(posbB[:], po_b[0:64, :])
                    bcA = norm_pool.tile([64, 512], F32, tag="bcA")
                    nc.gpsimd.partition_broadcast(bcA[:], rcfa[:])
                    bcB = norm_pool.tile([64, 512], F32, tag="bcB")
                    nc.gpsimd.partition_broadcast(bcB[:], rcfb[:])
                    nc.vector.tensor_mul(attn_c[0:64, hp, :], posbA[:], bcA[:])
                    nc.vector.tensor_mul(attn_c[64:128, hp, :], posbB[:], bcB[:])

            def outproj(c, attn_c):
                for j in range(4):
                    pw = [po_ps.tile([P, 512], F32, tag="po", name=f"pw{c}_{j}_{i}")
                          for i in range(2)]
                    for ci in range(4):
                        for oc in range(2):
                            nc.tensor.matmul(pw[oc][:],
                                             attn_c[:, ci, j * P:(j + 1) * P],
                                             wo_t[:, ci, oc * 512:(oc + 1) * 512],
                                             start=(ci == 0), stop=(ci == 3))
                    row = (4 * c + j) * P
                    for oc in range(2):
                        ot = out_pool.tile([P, 512], F32, tag="ot")
                        nc.vector.tensor_copy(ot[:], pw[oc][:])
                        nc.sync.dma_start(out[row:row + P, oc * 512:(oc + 1) * 512],
                                          ot[:])

            # ---- program: chunk 0 projections, then per chunk: attention,
            # next-chunk projections (scheduler filler for the PE), out-proj.
            proj_qk(0, qsb0, ksb0)
            proj_v(0, vsb0)
            for c in range(4):
                attn_c = attnc_pool.tile([P, 4, 512], F16, tag="attn",
                                         name=f"attn{c}")
                attn_chunk(c, attn_c)
                if c < 3:
                    qsb, ksb = load_qk(c + 1)
                    vsb = load_v(c + 1)
                    proj_qk(c + 1, qsb, ksb)
                    proj_v(c + 1, vsb)
                outproj(c, attn_c)
    nc.compile()
    return nc


def _tables():
    inv = (1.0 / (ROPE_BASE ** (np.arange(0, Dh, 2, dtype=np.float32) / Dh))
           ).astype(np.float32)                      # [32]
    pos = np.arange(S, dtype=np.float32)
    ang = pos[:, None] * inv[None, :]                # [S, 32]
    cos = np.cos(ang).astype(np.float32)
    sin = np.sin(ang).astype(np.float32)
    d = np.arange(P) % Dh
    i = d // 2
    cosf = np.ascontiguousarray(cos[:, i].T).astype(np.float16)   # [128, S]
    sgn = np.where(d % 2 == 0, 1.0, -1.0).astype(np.float32)
    sinf = np.ascontiguousarray(sin[:, i].T * sgn[:, None]).astype(np.float16)

    p = np.arange(P)
    j = np.arange(P)
    maskA = np.where(p[:, None] <= j[None, :], 1.0, 0.0).astype(np.float16)
    return cosf, sinf, maskA


def kernel(q, k, v, W_q, W_k, W_v, W_o):
    global _nc_cache, LAST_RESULT
    if _nc_cache is None:
        _nc_cache = _build_nc()
    nc = _nc_cache

    cosf, sinf, maskA = _tables()
    q = np.asarray(q, dtype=np.float32)
    k = np.asarray(k, dtype=np.float32)
    v = np.asarray(v, dtype=np.float32)
    W_q = np.asarray(W_q, dtype=np.float32)
    W_k = np.asarray(W_k, dtype=np.float32)
    W_v = np.asarray(W_v, dtype=np.float32)
    W_o = np.asarray(W_o, dtype=np.float32)

    in_maps = []
    for b in range(B):
        qTb = np.ascontiguousarray(q[b].T).astype(np.float16)
        kTb = np.ascontiguousarray(k[b].T).astype(np.float16)
        vTb = np.ascontiguousarray(v[b].T).astype(np.float16)
        for g in range(2):
            fs = slice(g * F, (g + 1) * F)
            in_maps.append({
                "qT": qTb, "kT": kTb, "vT": vTb,
                "wqT": np.ascontiguousarray(W_q[fs, :].T).astype(np.float16),
                "wkT": np.ascontiguousarray(W_k[fs, :].T).astype(np.float16),
                "wvT": np.ascontiguousarray(W_v[fs, :].T).astype(np.float16),
                "woT": np.ascontiguousarray(W_o[:, fs].T).astype(np.float16),
                "cosf": cosf, "sinf": sinf, "maskA": maskA,
            })

    res = bass_utils.run_bass_kernel_spmd(
        nc, in_maps, core_ids=list(range(N_CORES)), trace=KERNEL_TRACE)
    LAST_RESULT = res

    final = np.empty((B, S, D), dtype=np.float32)
    for b in range(B):
        final[b] = res.results[2 * b]["out"] + res.results[2 * b + 1]["out"]
    return final
